# revision 7
# baseline (speedup 1.0000x reference)
"""Trainium2 Bass kernel for nn_NGCF_18657337934509 (NGCF-style GNN + huge prediction matmul).

Design (8 NeuronCores, SPMD, one program + per-core input slabs):
  * Message passing is reformulated as dense matmuls against the 4096x4096
    edge-multiplicity matrix (exact, including duplicate edges).  Each core owns
    a 512-destination-node slab of every aggregation; layer outputs are
    AllGathered on-chip between layers.
  * BatchNorm batch stats are computed per-slab and AllReduced (4 KB);
    GCN biases are dropped (training-mode BN cancels them exactly).
  * GAT softmax is computed without the max-subtraction (mathematically
    identical; logits are O(10)), exploiting the rank-1 structure
    e[src,dst] = lrelu(s[src] + d[dst]) to build attention tiles densely.
  * The [4096,256]x[256,41476] prediction matmul is column-sharded; each core
    computes out^T[5185, 4096] so the per-class bias is per-partition (fused
    into PSUM eviction) and the 85 MB/core output write is fully contiguous.

Host-side work is index preprocessing (edge list -> dense multiplicity slabs,
degree counts) and array layout/slicing only; all floating-point NN compute
(embedding gather included) runs on device.
"""

import numpy as np

import concourse.bacc as bacc
import concourse.mybir as mybir
import concourse.tile as tile
from concourse.bass import IndirectOffsetOnAxis
from concourse.bass_utils import run_bass_kernel_spmd
from concourse.masks import make_identity

F32 = mybir.dt.float32
I32 = mybir.dt.int32
AF = mybir.ActivationFunctionType
ALU = mybir.AluOpType
AX = mybir.AxisListType

N = 4096            # nodes
D0 = 512            # concat feature dim
D = 256             # hidden dim
NCLS = 41476        # prediction classes
NCORES = 8
SLAB = N // NCORES  # 512 dest nodes per core
PCLS = 5185         # classes per core (8*5185 = 41480, zero padded)
PCLS_PAD = 5248     # 41*128 for per-partition bias packing
P = 128
NKT = N // P        # 32 src tiles
EPS = 1e-5


def build_bass():
    nc = bacc.Bacc(None, num_devices=NCORES)

    def inp(name, shape, dtype=F32):
        return nc.declare_dram_parameter(name, list(shape), dtype, isOutput=False)

    user_emb = inp("user_emb", [100000, D])
    item_emb = inp("item_emb", [50000, D])
    uidx = inp("uidx", [P, NKT], I32)
    iidx = inp("iidx", [P, NKT], I32)
    dinv_g_p = inp("dinv_g_p", [P, NKT])
    gcn0_w = inp("gcn0_w", [D0, D0])
    gcn1_w = inp("gcn1_w", [D0, D])
    bn_g = [inp("bn0_g", [D0, 1]), inp("bn1_g", [D, 1])]
    bn_b = [inp("bn0_b", [D0, 1]), inp("bn1_b", [D, 1])]
    sage_wl = inp("sage_wl", [D, D])
    sage_wr = inp("sage_wr", [D, D])
    sage_bl_bc = inp("sage_bl_bc", [P, D])
    cheb_w0 = inp("cheb_w0", [D, D])
    cheb_w1 = inp("cheb_w1", [D, D])
    cheb_b_bc = inp("cheb_b_bc", [P, D])
    gat_w = [inp("gat0_w", [D, D]), inp("gat1_w", [D, D])]
    gat_wT = [inp("gat0_wT", [D, D]), inp("gat1_wT", [D, D])]
    gat_as = [inp("gat0_as", [D, 1]), inp("gat1_as", [D, 1])]
    gat_ad = [inp("gat0_ad", [D, 1]), inp("gat1_ad", [D, 1])]
    gat_b = [inp("gat0_b", [D, 1]), inp("gat1_b", [D, 1])]
    ct_sl = inp("ct_sl", [N, SLAB])
    ct_nosl = inp("ct_nosl", [N, SLAB])
    cnt_row = inp("cnt_row", [1, SLAB])
    dinv_g_row = inp("dinv_g_row", [1, SLAB])
    dinv_g_slab_p = inp("dinv_g_slab_p", [P, SLAB // P])
    dinv_c_slab_p = inp("dinv_c_slab_p", [P, SLAB // P])
    dinv_c_row_neg = inp("dinv_c_row_neg", [1, SLAB])
    pred_w_slab = inp("pred_w_slab", [D, PCLS])
    pred_b_p = inp("pred_b_p", [P, PCLS_PAD // P])

    out_t = nc.declare_dram_parameter("out_t", [PCLS, N], F32, isOutput=True)

    groups = [list(range(NCORES))]

    with tile.TileContext(nc) as tc:
        psum_cm = tc.tile_pool(name="psum", bufs=8, space="PSUM")
        psum = psum_cm.__enter__()
        dram_cm = tc.tile_pool(name="dram", bufs=1, space="DRAM")
        dram = dram_cm.__enter__()
        const_cm = tc.tile_pool(name="const", bufs=1)
        const = const_cm.__enter__()
        slabs_cm = tc.tile_pool(name="slabs", bufs=1)
        slabs = slabs_cm.__enter__()
        wts_cm = tc.tile_pool(name="wts", bufs=1)
        wts = wts_cm.__enter__()
        cts_cm = tc.tile_pool(name="cts", bufs=4)
        cts = cts_cm.__enter__()
        xbig_cm = tc.tile_pool(name="xbig", bufs=34)
        xbig = xbig_cm.__enter__()

        def ptile(shape, name):
            return psum.tile(shape, F32, name=name, tag="ps")

        identity = const.tile([P, P], F32, name="identity")
        make_identity(nc, identity[:])
        ones_row = const.tile([1, P], F32, name="ones_row")
        nc.vector.memset(ones_row[:], 1.0)
        ones_col = const.tile([P, 1], F32, name="ones_col")
        nc.vector.memset(ones_col[:], 1.0)
        eps_col = const.tile([P, 1], F32, name="eps_col")
        nc.vector.memset(eps_col[:], EPS)

        def bcast_row(pool, src_row_ap, width, name):
            """[1, width] -> [128, width] (replicate across partitions)."""
            ps = ptile([P, width], name + "_ps")
            nc.tensor.matmul(ps[:], lhsT=ones_row[:], rhs=src_row_ap,
                             start=True, stop=True)
            out = pool.tile([P, width], F32, name=name)
            nc.scalar.copy(out[:], ps[:])
            return out

        def load_tiles(pool, dram_ap, rows, cols, name, tag=""):
            ts = []
            for i in range(rows // P):
                kw = {"tag": tag} if tag else {}
                t = pool.tile([P, cols], F32, name=f"{name}_{i}", **kw)
                nc.sync.dma_start(out=t[:], in_=dram_ap[i * P:(i + 1) * P, :])
                ts.append(t)
            return ts

        def load_vcols(pool, dram_ap, rows, name):
            ts = []
            for i in range(rows // P):
                t = pool.tile([P, 1], F32, name=f"{name}_{i}")
                nc.sync.dma_start(out=t[:], in_=dram_ap[i * P:(i + 1) * P, :])
                ts.append(t)
            return ts

        def transpose_tiles(pool, in_tiles, in_rows, in_cols, name):
            """in: [128, in_cols] tiles spanning [in_rows, in_cols]; out:
            [128, in_rows] tiles spanning the transpose."""
            outs = []
            for j in range(in_cols // P):
                t = pool.tile([P, in_rows], F32, name=f"{name}_{j}")
                for i in range(in_rows // P):
                    ps = ptile([P, P], f"{name}_tp")
                    nc.tensor.transpose(ps[:], in_tiles[i][:, j * P:(j + 1) * P],
                                        identity[:])
                    nc.scalar.copy(t[:, i * P:(i + 1) * P], ps[:])
                outs.append(t)
            return outs

        def allgather(slab_tiles, rows_per_tile, cols, name):
            nrows = rows_per_tile * len(slab_tiles)
            ag_in = dram.tile([nrows, cols], F32, name=f"agin_{name}")
            for i, t in enumerate(slab_tiles):
                nc.sync.dma_start(
                    out=ag_in[i * rows_per_tile:(i + 1) * rows_per_tile, :],
                    in_=t[:rows_per_tile, :])
            ag_out = dram.tile([NCORES * nrows, cols], F32,
                               name=f"agout_{name}", addr_space="Shared")
            nc.gpsimd.collective_compute(
                "AllGather", ALU.bypass, replica_groups=groups,
                ins=[ag_in[:].opt()], outs=[ag_out[:].opt()])
            return ag_out

        # constants / small vectors -----------------------------------------
        dg_row_s = const.tile([1, SLAB], F32, name="dg_row_s")
        nc.sync.dma_start(out=dg_row_s[:], in_=dinv_g_row[:])
        dg_bc = bcast_row(const, dg_row_s[:], SLAB, "dg_bc")
        dc_row_s = const.tile([1, SLAB], F32, name="dc_row_s")
        nc.sync.dma_start(out=dc_row_s[:], in_=dinv_c_row_neg[:])
        dc_bc = bcast_row(const, dc_row_s[:], SLAB, "dc_bc")
        cnt_row_s = const.tile([1, SLAB], F32, name="cnt_row_s")
        nc.sync.dma_start(out=cnt_row_s[:], in_=cnt_row[:])
        rcnt_row = const.tile([1, SLAB], F32, name="rcnt_row")
        nc.vector.reciprocal(rcnt_row[:], cnt_row_s[:])
        rcnt_bc = bcast_row(const, rcnt_row[:], SLAB, "rcnt_bc")

        dinv_g_pt = const.tile([P, NKT], F32, name="dinv_g_pt")
        nc.sync.dma_start(out=dinv_g_pt[:], in_=dinv_g_p[:])
        dg_slab_pt = const.tile([P, SLAB // P], F32, name="dg_slab_pt")
        nc.sync.dma_start(out=dg_slab_pt[:], in_=dinv_g_slab_p[:])
        dc_slab_pt = const.tile([P, SLAB // P], F32, name="dc_slab_pt")
        nc.sync.dma_start(out=dc_slab_pt[:], in_=dinv_c_slab_p[:])
        uidx_t = const.tile([P, NKT], I32, name="uidx_t")
        nc.sync.dma_start(out=uidx_t[:], in_=uidx[:])
        iidx_t = const.tile([P, NKT], I32, name="iidx_t")
        nc.sync.dma_start(out=iidx_t[:], in_=iidx[:])

        bn_g_t = [load_vcols(const, bn_g[0][:], D0, "bn0g"),
                  load_vcols(const, bn_g[1][:], D, "bn1g")]
        bn_b_t = [load_vcols(const, bn_b[0][:], D0, "bn0b"),
                  load_vcols(const, bn_b[1][:], D, "bn1b")]

        # weights -----------------------------------------------------------
        gcn_w_t = [load_tiles(wts, gcn0_w[:], D0, D0, "g0w"),
                   load_tiles(wts, gcn1_w[:], D0, D, "g1w")]
        sage_wl_t = load_tiles(wts, sage_wl[:], D, D, "swl")
        sage_wr_t = load_tiles(wts, sage_wr[:], D, D, "swr")
        cheb_w0_t = load_tiles(wts, cheb_w0[:], D, D, "cw0")
        cheb_w1_t = load_tiles(wts, cheb_w1[:], D, D, "cw1")
        sage_bl_t = wts.tile([P, D], F32, name="sage_bl_t")
        nc.sync.dma_start(out=sage_bl_t[:], in_=sage_bl_bc[:])
        cheb_b_t = wts.tile([P, D], F32, name="cheb_b_t")
        nc.sync.dma_start(out=cheb_b_t[:], in_=cheb_b_bc[:])
        gat_w_t = [load_tiles(wts, gat_w[i][:], D, D, f"gw{i}") for i in range(2)]
        gat_wT_t = [load_tiles(wts, gat_wT[i][:], D, D, f"gwT{i}") for i in range(2)]
        gat_as_t = [load_vcols(wts, gat_as[i][:], D, f"gas{i}") for i in range(2)]
        gat_ad_t = [load_vcols(wts, gat_ad[i][:], D, f"gad{i}") for i in range(2)]
        gat_b_t = [load_vcols(wts, gat_b[i][:], D, f"gb{i}") for i in range(2)]

        def ct_stream(dram_t, k, nm):
            t = cts.tile([P, SLAB], F32, name=f"ct_{nm}", tag="ct")
            nc.sync.dma_start(out=t[:], in_=dram_t[k * P:(k + 1) * P, :])
            return t

        # phase 0: embedding gather -> X0' = dinv_g * [U | I] ---------------
        x0 = []
        for i in range(NKT):
            t = xbig.tile([P, D0], F32, name=f"x0_{i}", tag="x")
            nc.gpsimd.indirect_dma_start(
                out=t[:, 0:D], out_offset=None, in_=user_emb[:],
                in_offset=IndirectOffsetOnAxis(ap=uidx_t[:, i:i + 1], axis=0))
            nc.gpsimd.indirect_dma_start(
                out=t[:, D:D0], out_offset=None, in_=item_emb[:],
                in_offset=IndirectOffsetOnAxis(ap=iidx_t[:, i:i + 1], axis=0))
            nc.vector.tensor_scalar_mul(t[:], t[:], dinv_g_pt[:, i:i + 1])
            x0.append(t)

        # GCN + BN + relu (x2) ----------------------------------------------
        def gcn_layer(x_tiles, feat_in, feat_out, li, xt_pool=None):
            ph_cm = tc.tile_pool(name=f"gcn{li}", bufs=1)
            ph = ph_cm.__enter__()
            if xt_pool is None:
                xt_pool = ph
            n_mi, n_mo = feat_in // P, feat_out // P

            # T[f_in, c] = sum_src x'[src, f_in] ct_sl[src, c]
            t_ps = [ptile([P, SLAB], f"T{li}_{m}") for m in range(n_mi)]
            for k in range(NKT):
                ctt = ct_stream(ct_sl, k, f"g{li}")
                for m in range(n_mi):
                    nc.tensor.matmul(t_ps[m][:],
                                     lhsT=x_tiles[k][:, m * P:(m + 1) * P],
                                     rhs=ctt[:],
                                     start=(k == 0), stop=(k == NKT - 1))
            t_sb = []
            for m in range(n_mi):
                t = ph.tile([P, SLAB], F32, name=f"tsb{li}_{m}")
                nc.scalar.copy(t[:], t_ps[m][:])
                t_sb.append(t)

            # aggT[f_out, c] = sum_f_in w[f_in, f_out] T[f_in, c]
            agg_ps = [ptile([P, SLAB], f"agg{li}_{m}") for m in range(n_mo)]
            for k in range(n_mi):
                for m in range(n_mo):
                    nc.tensor.matmul(agg_ps[m][:],
                                     lhsT=gcn_w_t[li][k][:, m * P:(m + 1) * P],
                                     rhs=t_sb[k][:],
                                     start=(k == 0), stop=(k == n_mi - 1))

            # u = aggT * dinv_g[dest]; partial stats over own slab
            u_t, s_t, q_t = [], [], []
            for m in range(n_mo):
                u = ph.tile([P, SLAB], F32, name=f"u{li}_{m}")
                nc.vector.tensor_tensor(out=u[:], in0=agg_ps[m][:], in1=dg_bc[:],
                                        op=ALU.mult)
                s = ph.tile([P, 1], F32, name=f"s{li}_{m}")
                nc.vector.tensor_reduce(out=s[:], in_=u[:], axis=AX.X, op=ALU.add)
                scratch = ph.tile([P, SLAB], F32, name=f"scr{li}",
                                  tag=f"scr{li}", bufs=2)
                q = ph.tile([P, 1], F32, name=f"q{li}_{m}")
                nc.scalar.activation(scratch[:], u[:], AF.Square, accum_out=q[:])
                u_t.append(u)
                s_t.append(s)
                q_t.append(q)

            ar_in = dram.tile([feat_out, 2], F32, name=f"arin{li}")
            for m in range(n_mo):
                nc.sync.dma_start(out=ar_in[m * P:(m + 1) * P, 0:1], in_=s_t[m][:])
                nc.sync.dma_start(out=ar_in[m * P:(m + 1) * P, 1:2], in_=q_t[m][:])
            ar_out = dram.tile([feat_out, 2], F32, name=f"arout{li}",
                               addr_space="Shared")
            nc.gpsimd.collective_compute(
                "AllReduce", ALU.add, replica_groups=groups,
                ins=[ar_in[:].opt()], outs=[ar_out[:].opt()])

            # x^T = relu(u*A + B), A = g*rstd, B = beta - mean*A
            xt_t = []
            for m in range(n_mo):
                sq = ph.tile([P, 2], F32, name=f"sq{li}_{m}")
                nc.sync.dma_start(out=sq[:], in_=ar_out[m * P:(m + 1) * P, :])
                mean = ph.tile([P, 1], F32, name=f"mean{li}_{m}")
                nc.vector.tensor_scalar_mul(mean[:], sq[:, 0:1], 1.0 / N)
                msq = ph.tile([P, 1], F32, name=f"msq{li}_{m}")
                nc.vector.tensor_tensor(out=msq[:], in0=mean[:], in1=mean[:],
                                        op=ALU.mult)
                var = ph.tile([P, 1], F32, name=f"var{li}_{m}")
                nc.vector.scalar_tensor_tensor(
                    out=var[:], in0=sq[:, 1:2], scalar=1.0 / N, in1=msq[:],
                    op0=ALU.mult, op1=ALU.subtract)
                sd = ph.tile([P, 1], F32, name=f"sd{li}_{m}")
                nc.scalar.activation(sd[:], var[:], AF.Sqrt, bias=eps_col[:])
                rstd = ph.tile([P, 1], F32, name=f"rstd{li}_{m}")
                nc.vector.reciprocal(rstd[:], sd[:])
                a_c = ph.tile([P, 1], F32, name=f"ac{li}_{m}")
                nc.vector.tensor_tensor(out=a_c[:], in0=bn_g_t[li][m][:],
                                        in1=rstd[:], op=ALU.mult)
                nega = ph.tile([P, 1], F32, name=f"nega{li}_{m}")
                nc.vector.tensor_scalar_mul(nega[:], a_c[:], -1.0)
                b_c = ph.tile([P, 1], F32, name=f"bc{li}_{m}")
                nc.vector.scalar_tensor_tensor(
                    out=b_c[:], in0=mean[:], scalar=nega[:], in1=bn_b_t[li][m][:],
                    op0=ALU.mult, op1=ALU.add)
                xt = xt_pool.tile([P, SLAB], F32, name=f"xt{li}_{m}")
                nc.scalar.activation(xt[:], u_t[m][:], AF.Relu,
                                     bias=b_c[:], scale=a_c[:])
                xt_t.append(xt)

            xs_t = transpose_tiles(ph, xt_t, feat_out, SLAB, f"xs{li}")
            if li == 0:
                for j, t in enumerate(xs_t):
                    nc.vector.tensor_scalar_mul(t[:], t[:], dg_slab_pt[:, j:j + 1])
            ag = allgather(xs_t, P, feat_out, f"x{li + 1}")
            x_next = load_tiles(xbig, ag[:], N, feat_out, f"x{li + 1}", tag="x")
            ph_cm.__exit__(None, None, None)
            return x_next, xt_t

        x1, _ = gcn_layer(x0, D0, D0, 0)
        x2, x2T_slab = gcn_layer(x1, D0, D, 1, xt_pool=slabs)

        # SAGE + relu --------------------------------------------------------
        sg_cm = tc.tile_pool(name="sage", bufs=1)
        sg = sg_cm.__enter__()
        ms_ps = [ptile([P, SLAB], f"ms_{m}") for m in range(D // P)]
        for k in range(NKT):
            ctt = ct_stream(ct_nosl, k, "sg")
            for m in range(D // P):
                nc.tensor.matmul(ms_ps[m][:], lhsT=x2[k][:, m * P:(m + 1) * P],
                                 rhs=ctt[:], start=(k == 0), stop=(k == NKT - 1))
        meanT = []
        for m in range(D // P):
            t = sg.tile([P, SLAB], F32, name=f"meanT_{m}")
            nc.vector.tensor_tensor(out=t[:], in0=ms_ps[m][:], in1=rcnt_bc[:],
                                    op=ALU.mult)
            meanT.append(t)
        x3_slab = []
        for m in range(SLAB // P):
            ps = ptile([P, D], f"sgo_{m}")
            for k in range(D // P):
                nc.tensor.matmul(ps[:], lhsT=meanT[k][:, m * P:(m + 1) * P],
                                 rhs=sage_wl_t[k][:], start=(k == 0), stop=False)
            for k in range(D // P):
                nc.tensor.matmul(ps[:], lhsT=x2T_slab[k][:, m * P:(m + 1) * P],
                                 rhs=sage_wr_t[k][:], start=False,
                                 stop=(k == D // P - 1))
            tmp = sg.tile([P, D], F32, name=f"sgt_{m}", tag="sgt", bufs=2)
            nc.vector.tensor_tensor(out=tmp[:], in0=ps[:], in1=sage_bl_t[:],
                                    op=ALU.add)
            x3 = sg.tile([P, D], F32, name=f"x3s_{m}")
            nc.scalar.activation(x3[:], tmp[:], AF.Relu)
            x3_slab.append(x3)
        # x3T slab (cheb root) + scaled x3 for allgather
        x3T_slab = transpose_tiles(slabs, x3_slab, SLAB, D, "x3T")
        x3sc = []
        for j, t in enumerate(x3_slab):
            ts_ = sg.tile([P, D], F32, name=f"x3sc_{j}")
            nc.vector.tensor_scalar_mul(ts_[:], t[:], dc_slab_pt[:, j:j + 1])
            x3sc.append(ts_)
        ag3 = allgather(x3sc, P, D, "x3")
        x3f = load_tiles(xbig, ag3[:], N, D, "x3f", tag="x")
        sg_cm.__exit__(None, None, None)

        # Cheb (K=2, sym norm) + relu ---------------------------------------
        cb_cm = tc.tile_pool(name="cheb", bufs=1)
        cb = cb_cm.__enter__()
        tx_ps = [ptile([P, SLAB], f"tx_{m}") for m in range(D // P)]
        for k in range(NKT):
            ctt = ct_stream(ct_nosl, k, "cb")
            for m in range(D // P):
                nc.tensor.matmul(tx_ps[m][:], lhsT=x3f[k][:, m * P:(m + 1) * P],
                                 rhs=ctt[:], start=(k == 0), stop=(k == NKT - 1))
        tx1T = []
        for m in range(D // P):
            t = cb.tile([P, SLAB], F32, name=f"tx1T_{m}")
            nc.vector.tensor_tensor(out=t[:], in0=tx_ps[m][:], in1=dc_bc[:],
                                    op=ALU.mult)
            tx1T.append(t)
        x4_slab = []
        for m in range(SLAB // P):
            ps = ptile([P, D], f"cbo_{m}")
            for k in range(D // P):
                nc.tensor.matmul(ps[:], lhsT=tx1T[k][:, m * P:(m + 1) * P],
                                 rhs=cheb_w1_t[k][:], start=(k == 0), stop=False)
            for k in range(D // P):
                nc.tensor.matmul(ps[:], lhsT=x3T_slab[k][:, m * P:(m + 1) * P],
                                 rhs=cheb_w0_t[k][:], start=False,
                                 stop=(k == D // P - 1))
            tmp = cb.tile([P, D], F32, name=f"cbt_{m}", tag="cbt", bufs=2)
            nc.vector.tensor_tensor(out=tmp[:], in0=ps[:], in1=cheb_b_t[:],
                                    op=ALU.add)
            x4 = cb.tile([P, D], F32, name=f"x4s_{m}")
            nc.scalar.activation(x4[:], tmp[:], AF.Relu)
            x4_slab.append(x4)
        xT_slab = transpose_tiles(slabs, x4_slab, SLAB, D, "x4T")
        cb_cm.__exit__(None, None, None)
        ag = allgather(xT_slab, P, SLAB, "x4T")
        xbig_cm.__exit__(None, None, None)

        # GAT x2 (+ elu) -----------------------------------------------------
        for li in range(2):
            g_cm = tc.tile_pool(name=f"gat{li}", bufs=1)
            g = g_cm.__enter__()
            xT = [g.tile([P, N], F32, name=f"xT{li}_{p}") for p in range(D // P)]
            for p in range(D // P):
                for k in range(NCORES):
                    nc.sync.dma_start(
                        out=xT[p][:, k * SLAB:(k + 1) * SLAB],
                        in_=ag[k * D + p * P:k * D + (p + 1) * P, :])

            # xw full [4096, 256]
            xw = []
            for m in range(NKT):
                ps = ptile([P, D], f"xw{li}_ps")
                for k in range(D // P):
                    nc.tensor.matmul(ps[:], lhsT=xT[k][:, m * P:(m + 1) * P],
                                     rhs=gat_w_t[li][k][:],
                                     start=(k == 0), stop=(k == D // P - 1))
                t = g.tile([P, D], F32, name=f"xw{li}_{m}")
                nc.scalar.copy(t[:], ps[:])
                xw.append(t)

            # wa_s, wa_d = w @ a  [256, 1]
            wa = {}
            for nm, av in (("s", gat_as_t[li]), ("d", gat_ad_t[li])):
                cols = []
                for m in range(D // P):
                    ps = ptile([P, 1], f"wa{nm}{li}_ps")
                    for k in range(D // P):
                        nc.tensor.matmul(
                            ps[:], lhsT=gat_wT_t[li][k][:, m * P:(m + 1) * P],
                            rhs=av[k][:], start=(k == 0), stop=(k == D // P - 1))
                    t = g.tile([P, 1], F32, name=f"wa{nm}{li}_{m}")
                    nc.vector.tensor_copy(t[:], ps[:])
                    cols.append(t)
                wa[nm] = cols

            # s[src] per src tile
            s_col = []
            for m in range(NKT):
                ps = ptile([P, 1], f"sc{li}_ps")
                for k in range(D // P):
                    nc.tensor.matmul(ps[:], lhsT=xT[k][:, m * P:(m + 1) * P],
                                     rhs=wa["s"][k][:],
                                     start=(k == 0), stop=(k == D // P - 1))
                t = g.tile([P, 1], F32, name=f"sc{li}_{m}")
                nc.vector.tensor_copy(t[:], ps[:])
                s_col.append(t)

            # d over own dest slab -> broadcast
            dps = ptile([1, SLAB], f"d{li}_ps")
            for k in range(D // P):
                nc.tensor.matmul(dps[:], lhsT=wa["d"][k][:], rhs=xT_slab[k][:],
                                 start=(k == 0), stop=(k == D // P - 1))
            d_row = g.tile([1, SLAB], F32, name=f"d{li}_row")
            nc.scalar.copy(d_row[:], dps[:])
            d_bc = bcast_row(g, d_row[:], SLAB, f"d{li}_bc")

            # attention tiles + U^T + denom
            u_ps = [ptile([P, SLAB], f"U{li}_{m}") for m in range(D // P)]
            den_ps = ptile([1, SLAB], f"den{li}_ps")
            for k in range(NKT):
                ctt = ct_stream(ct_sl, k, f"ga{li}")
                e1 = g.tile([P, SLAB], F32, name=f"e1{li}", tag=f"e1{li}", bufs=3)
                nc.scalar.activation(e1[:], d_bc[:], AF.Lrelu,
                                     bias=s_col[k][:], alpha=0.2)
                e2 = g.tile([P, SLAB], F32, name=f"e2{li}", tag=f"e2{li}", bufs=3)
                nc.scalar.activation(e2[:], e1[:], AF.Exp)
                wt_ = g.tile([P, SLAB], F32, name=f"wt{li}", tag=f"wt{li}", bufs=3)
                nc.vector.tensor_tensor(out=wt_[:], in0=e2[:], in1=ctt[:],
                                        op=ALU.mult)
                for m in range(D // P):
                    nc.tensor.matmul(u_ps[m][:], lhsT=xw[k][:, m * P:(m + 1) * P],
                                     rhs=wt_[:], start=(k == 0),
                                     stop=(k == NKT - 1))
                nc.tensor.matmul(den_ps[:], lhsT=ones_col[:], rhs=wt_[:],
                                 start=(k == 0), stop=(k == NKT - 1))

            den_row = g.tile([1, SLAB], F32, name=f"den{li}_row")
            nc.vector.reciprocal(den_row[:], den_ps[:])
            rden_bc = bcast_row(g, den_row[:], SLAB, f"rden{li}_bc")

            # out^T slab = elu(U/denom + b) into the cross-phase slabs pool
            new_slab = []
            for m in range(D // P):
                u = g.tile([P, SLAB], F32, name=f"uo{li}_{m}")
                nc.vector.tensor_tensor(out=u[:], in0=u_ps[m][:], in1=rden_bc[:],
                                        op=ALU.mult)
                z = g.tile([P, SLAB], F32, name=f"z{li}_{m}")
                nc.vector.tensor_scalar_add(z[:], u[:], gat_b_t[li][m][:])
                zm = g.tile([P, SLAB], F32, name=f"zm{li}_{m}")
                nc.vector.tensor_scalar(zm[:], u[:], gat_b_t[li][m][:], 0.0,
                                        ALU.add, ALU.min)
                ez = g.tile([P, SLAB], F32, name=f"ez{li}_{m}")
                nc.scalar.activation(ez[:], zm[:], AF.Exp)
                o = slabs.tile([P, SLAB], F32, name=f"xT{5 + li}_{m}")
                nc.vector.scalar_tensor_tensor(
                    out=o[:], in0=ez[:], scalar=-1.0, in1=z[:],
                    op0=ALU.add, op1=ALU.max)
                new_slab.append(o)
            xT_slab = new_slab
            ag = allgather(xT_slab, P, SLAB, f"x{5 + li}T")
            g_cm.__exit__(None, None, None)

        # prediction: out^T[class, node] = pred_w_slab^T x6 + b --------------
        cts_cm.__exit__(None, None, None)
        wts_cm.__exit__(None, None, None)
        pr_cm = tc.tile_pool(name="pred", bufs=1)
        pr = pr_cm.__enter__()
        x6T = [pr.tile([P, N], F32, name=f"x6T_{p}") for p in range(D // P)]
        for p in range(D // P):
            for k in range(NCORES):
                nc.sync.dma_start(
                    out=x6T[p][:, k * SLAB:(k + 1) * SLAB],
                    in_=ag[k * D + p * P:k * D + (p + 1) * P, :])
        pw_t = load_tiles(pr, pred_w_slab[:], D, PCLS, "pw")
        pb_t = pr.tile([P, PCLS_PAD // P], F32, name="pb_t")
        nc.sync.dma_start(out=pb_t[:], in_=pred_b_p[:])

        NMT = PCLS_PAD // P  # 41
        NCH = N // 512       # 8
        for mt in range(NMT):
            msz = min(P, PCLS - mt * P)  # 128, last 65
            stage = pr.tile([P, N], F32, name="stage", tag="stage", bufs=3)
            for ch in range(NCH):
                ps = ptile([msz, 512], "pred_ps")
                for k in range(D // P):
                    nc.tensor.matmul(
                        ps[:], lhsT=pw_t[k][:, mt * P:mt * P + msz],
                        rhs=x6T[k][:, ch * 512:(ch + 1) * 512],
                        start=(k == 0), stop=(k == D // P - 1))
                dst = stage[:msz, ch * 512:(ch + 1) * 512]
                if (mt + ch) % 2 == 0:
                    nc.scalar.activation(dst, ps[:], AF.Identity,
                                         bias=pb_t[:msz, mt:mt + 1])
                else:
                    nc.vector.tensor_scalar_add(dst, ps[:], pb_t[:msz, mt:mt + 1])
            nc.sync.dma_start(out=out_t[mt * P:mt * P + msz, :],
                              in_=stage[:msz, :])
        pr_cm.__exit__(None, None, None)

        slabs_cm.__exit__(None, None, None)
        const_cm.__exit__(None, None, None)
        dram_cm.__exit__(None, None, None)
        psum_cm.__exit__(None, None, None)

    nc.finalize()
    return nc


_NC = None


def _get_nc():
    global _NC
    if _NC is None:
        _NC = build_bass()
    return _NC


# ---------------------------------------------------------------------------
# host entry
# ---------------------------------------------------------------------------

def _prep_inputs(inputs):
    edge_index = np.asarray(inputs["edge_index"])
    row = edge_index[0].astype(np.int64)
    col = edge_index[1].astype(np.int64)
    ct = np.bincount(row * N + col, minlength=N * N).astype(np.float32).reshape(N, N)
    ct_nosl = ct.copy()
    ct_sl = ct
    ct_sl[np.arange(N), np.arange(N)] += 1.0
    deg_g = ct_sl.sum(axis=0)
    dinv_g = (deg_g.astype(np.float32) ** -0.5).astype(np.float32)
    deg_c = ct_nosl.sum(axis=1)
    with np.errstate(divide="ignore"):
        dinv_c = np.where(deg_c > 0, deg_c.astype(np.float32) ** -0.5, 0.0)
    dinv_c = dinv_c.astype(np.float32)
    cnt = np.maximum(ct_nosl.sum(axis=0), 1.0).astype(np.float32)

    f32 = lambda x: np.ascontiguousarray(np.asarray(x), dtype=np.float32)
    col_v = lambda x: f32(x).reshape(-1, 1)
    pack = lambda v: np.ascontiguousarray(np.asarray(v, np.float32).reshape(-1, P).T)

    pred_w = f32(inputs["pred_w"])
    pred_b = f32(inputs["pred_b"])
    pw_pad = np.zeros((D, NCORES * PCLS), np.float32)
    pw_pad[:, :NCLS] = pred_w
    pb_pad = np.zeros(NCORES * PCLS, np.float32)
    pb_pad[:NCLS] = pred_b

    common = {
        "user_emb": f32(inputs["user_emb"]),
        "item_emb": f32(inputs["item_emb"]),
        "uidx": np.ascontiguousarray(
            np.asarray(inputs["user_idx"], np.int64).astype(np.int32)
            .reshape(-1, P).T),
        "iidx": np.ascontiguousarray(
            np.asarray(inputs["item_idx"], np.int64).astype(np.int32)
            .reshape(-1, P).T),
        "dinv_g_p": pack(dinv_g),
        "gcn0_w": f32(inputs["gcn0_w"]),
        "gcn1_w": f32(inputs["gcn1_w"]),
        "bn0_g": col_v(inputs["bn0_g"]), "bn0_b": col_v(inputs["bn0_b"]),
        "bn1_g": col_v(inputs["bn1_g"]), "bn1_b": col_v(inputs["bn1_b"]),
        "sage_wl": f32(inputs["sage_wl"]), "sage_wr": f32(inputs["sage_wr"]),
        "sage_bl_bc": np.tile(f32(inputs["sage_bl"]).reshape(1, -1), (P, 1)),
        "cheb_w0": f32(inputs["cheb_w0"]), "cheb_w1": f32(inputs["cheb_w1"]),
        "cheb_b_bc": np.tile(f32(inputs["cheb_b"]).reshape(1, -1), (P, 1)),
        "gat0_w": f32(inputs["gat0_w"]),
        "gat0_wT": np.ascontiguousarray(f32(inputs["gat0_w"]).T),
        "gat0_as": col_v(inputs["gat0_as"]), "gat0_ad": col_v(inputs["gat0_ad"]),
        "gat0_b": col_v(inputs["gat0_b"]),
        "gat1_w": f32(inputs["gat1_w"]),
        "gat1_wT": np.ascontiguousarray(f32(inputs["gat1_w"]).T),
        "gat1_as": col_v(inputs["gat1_as"]), "gat1_ad": col_v(inputs["gat1_ad"]),
        "gat1_b": col_v(inputs["gat1_b"]),
    }

    in_maps = []
    for k in range(NCORES):
        sl = slice(k * SLAB, (k + 1) * SLAB)
        pb_slab = np.zeros(PCLS_PAD, np.float32)
        pb_slab[:PCLS] = pb_pad[k * PCLS:(k + 1) * PCLS]
        m = dict(common)
        m.update({
            "ct_sl": np.ascontiguousarray(ct_sl[:, sl]),
            "ct_nosl": np.ascontiguousarray(ct_nosl[:, sl]),
            "cnt_row": cnt[sl].reshape(1, -1),
            "dinv_g_row": dinv_g[sl].reshape(1, -1),
            "dinv_g_slab_p": pack(dinv_g[sl]),
            "dinv_c_slab_p": pack(dinv_c[sl]),
            "dinv_c_row_neg": (-dinv_c[sl]).reshape(1, -1),
            "pred_w_slab": np.ascontiguousarray(pw_pad[:, k * PCLS:(k + 1) * PCLS]),
            "pred_b_p": pack(pb_slab),
        })
        in_maps.append(m)
    return in_maps


def run(inputs, trace=False):
    in_maps = _prep_inputs(inputs)
    nc = _get_nc()
    res = run_bass_kernel_spmd(nc, in_maps, list(range(NCORES)), trace=trace)
    out = np.empty((N, NCORES * PCLS), np.float32)
    for k in range(NCORES):
        out[:, k * PCLS:(k + 1) * PCLS] = res.results[k]["out_t"].T
    return out[:, :NCLS], res


def kernel(**inputs) -> np.ndarray:
    out, _ = run(inputs, trace=False)
    return out
